# revision 19
# baseline (speedup 1.0000x reference)
"""LoRA-MHSA Trainium2 kernel.

Data-parallel over batch B=8 (one sample per NeuronCore). The per-sample LoRA
adapters are folded into the base weights on the host (exact algebra:
x@(W + (a/r)B@A).T == x@W.T + (a/r)(x@A.T)@B.T), so the device kernel is a
plain MHSA with per-core weights:
  qkv = x @ Wqkv_eff.T + b_qkv ; 16-head SDPA over T=1024, dh=64 ;
  out = y @ Wp_eff.T + b_p

All matmul operands are bf16 (full PE rate, halves DMA + SBUF); PSUM
accumulation stays fp32. Biases are added during the PSUM->SBUF drain copies
(host-replicated bias tiles), costing no extra engine time.

Layout: activations channel-major ([C, T]) so q/k head slabs feed the scores
matmul directly. Softmax needs no max-subtraction: scores are O(1) by
construction.

PV is computed FLIPPED: stationary = exp(scores) block [128tk x 128tq],
moving = v [128tk x 64ch], so each PV matmul streams only 64 columns instead
of 512 (matmul cost is out-free-size cycles; stationary loads are free).
This cuts PV from 131072 to 66560 PE cycles (~27us). The softmax
denominators come from an extra N=1 matmul per group (moving = ones column).
The flipped output is [tq, ch]-major; after the per-partition-scalar
normalize (batched reciprocal + 8 tensor_scalar muls per half), one
DMA-transpose per head pair (64 xbar tiles, ~0.9us on the otherwise idle
DMA engines) restores the channel-major yt slab the projection needs.

PSUM budget (8 banks): qk build 2 + scores 2x2 + pv accum 1 + denom 1 = 8.

Schedule (CoreSim: ~224.8us baseline; PE floor now ~191.6us):
- x streams in 8 chunks on the SP DMA queue while all weights stream on the
  Pool queue, so the v-phase GEMM starts ~2us in and is never DMA-paced
  (each DMA has a 500ns descriptor-gen floor; one chunk per consumer step
  is the optimal granularity).
- build(0) runs in the v-phase PSUM pool (no pool-switch barrier); its
  k-part accumulates as three staggered groups (512/384/128 t-cols) so the
  pool-switch barrier's one exposed drain is a tiny 128-col ACT piece.
- The q/k build of head-pair hp is interleaved step-by-step with the
  attention of pair hp-1 (3-step build head-start) so Exp (ACT) hides under
  build matmuls. Builds accumulate tch-major: each 512-col half drains
  (ACT or DVE, alternating) while the other half accumulates, and each
  pair's q/k pieces live in independent SBUF tiles (dependency tracking is
  tile-granular), so no consumer ever waits on a drain chain.
- Scores for both heads of a pair land in one 2-bank PSUM group and are
  exponentiated by a single fused ACT instruction; ys drains to SBUF
  immediately to recycle its PSUM bank.
- The last pair's attention has no build to hide Exp under: the first output
  tile's ci 0..6 projection accumulation interleaves into it using the idle
  build PSUM banks.
- The projection (weights prefetched long before) runs inside the attention
  pool scope reusing the sps PSUM ring -- no pool-switch barrier -- and the
  last tile accumulates as three staggered groups (512/320/192 cols) whose
  drains use separate staging tiles and three DMA queues, so the final
  drain chain (add + store + fixed ~2.2us DMA-completion constants) starts
  at the earliest possible instant after the last matmul.
"""

import sys
from itertools import chain, zip_longest

sys.path.insert(0, "/opt/trn_rl_repo")

import numpy as np
import ml_dtypes

import concourse.bass as bass
import concourse.tile as tile
from concourse import bacc, mybir
from concourse.bass_utils import run_bass_kernel_spmd

T = 1024
C = 1024
C3 = 3072
H = 16
DH = 64
RANK = 8
ALPHA_OVER_RANK = 1.0 / 8.0
SM_SCALE = 0.125  # 1/sqrt(dh)
NCORES = 8

F32 = mybir.dt.float32
BF16 = mybir.dt.bfloat16
EXP = mybir.ActivationFunctionType.Exp
NPBF16 = ml_dtypes.bfloat16

ts = bass.ts

TT = T // 128     # 8 t tiles
TCH = T // 512    # 2 t chunks (psum free dim)
CINT = C // 128   # 8 contraction tiles
NPAIR = H // 2    # 8 head pairs


def _build():
    nc = bacc.Bacc("TRN2", target_bir_lowering=False, debug=False)

    xT_d = nc.dram_tensor("xT", [C, T], BF16, kind="ExternalInput")
    wqkvT_d = nc.dram_tensor("wqkvT", [C, C3], BF16, kind="ExternalInput")
    wpT_d = nc.dram_tensor("wpT", [C, C], BF16, kind="ExternalInput")
    bqk_d = nc.dram_tensor("bqk", [128, H], F32, kind="ExternalInput")
    bv_d = nc.dram_tensor("bv", [128, C], F32, kind="ExternalInput")
    bo_d = nc.dram_tensor("bo", [128, C], F32, kind="ExternalInput")
    out_d = nc.dram_tensor("out", [T, C], F32, kind="ExternalOutput")

    with tile.TileContext(nc) as tc:
      with tc.tile_pool(name="res", bufs=1) as res:
        xT = res.tile([128, CINT, T], BF16, tag="xT")
        # v split: heads 0-7 built in the prologue, heads 8-15 deferred as
        # PE filler inside attention windows 0..3 (they are not needed
        # until window 4)
        vvl = res.tile([128, TT, 8, DH], BF16, tag="vvl")
        vvh = res.tile([128, TT, 8, DH], BF16, tag="vvh")
        ones = res.tile([128, 1], BF16, tag="ones")
        yt = res.tile([128, CINT, T], BF16, tag="yt")
        wpa = res.tile([128, CINT, C], BF16, tag="wpa")
        # q/k weight slabs: [g] covers 4 consecutive 128-col parts.
        # g0: q cols 0-511 (pairs 0-3), g1: q 512-1023, g2: k 1024-1535,
        # g3: k 1536-2047.
        wqk = [
            res.tile([128, CINT, 512], BF16, tag=f"wqk{g}", name=f"wqk{g}")
            for g in range(4)
        ]
        bqk = res.tile([128, H], F32, tag="bqk")
        bv = res.tile([128, H, DH], F32, tag="bv")
        bo = res.tile([128, C], F32, tag="bo")

        # ---- DMA streams: x chunks on SP, all weights on Pool ----
        # first chunk arrives in 256-col slivers so the very first stationary
        # load (xT[:, 0, 0:128]) lands ~0.6us earlier
        for s in range(4):
            nc.sync.dma_start(
                out=xT[:, 0, ts(s, 256)], in_=xT_d[0:128, ts(s, 256)]
            )
        for ci in range(1, CINT):
            nc.sync.dma_start(out=xT[:, ci, :], in_=xT_d[ts(ci, 128), :])
        nc.sync.dma_start(out=bo[:], in_=bo_d[:])
        nc.sync.dma_start(out=bqk[:], in_=bqk_d[:])

        with tc.tile_pool(name="wvp", bufs=2) as wvp:
            wvv = []
            for cch in range(2):
                w = wvp.tile([128, CINT, 512], BF16, tag="wv", name=f"wvv{cch}")
                for ci in range(CINT):
                    nc.gpsimd.dma_start(
                        out=w[:, ci, :],
                        in_=wqkvT_d[
                            ts(ci, 128), 2048 + cch * 512 : 2560 + cch * 512
                        ],
                    )
                wvv.append(w)
            nc.gpsimd.dma_start(
                out=bv[:], in_=bv_d.rearrange("p (h d) -> p h d", d=DH)
            )
            for g in range(4):
                nc.gpsimd.dma_start(
                    out=wqk[g][:],
                    in_=wqkvT_d[:, ts(g, 512)].rearrange("(n p) c -> p n c", p=128),
                )
            nc.gpsimd.dma_start(
                out=wpa[:], in_=wpT_d.rearrange("(n p) c -> p n c", p=128)
            )

            nc.vector.memset(ones[:], 1.0)

            # ---- interleaved: qk build for pair hp + attention pair hp-1 ----
            with tc.tile_pool(name="qkpool", bufs=3) as qkpool, \
                 tc.tile_pool(name="att", bufs=3) as att:

                qktiles = {}

                def build_steps(hp_i, psum_pool, psum_tag):
                    # four independent [128, 512] tiles per pair (q/k x
                    # tch-half): dependency tracking is tile-granular, so
                    # separate tiles let the first scores run as soon as
                    # the halves they actually read are drained
                    qt = {
                        (part, tch): qkpool.tile(
                            [128, 512], BF16, tag=f"qk{part}{tch}", name="qk"
                        )
                        for part in range(2) for tch in range(TCH)
                    }
                    qktiles[hp_i] = qt
                    for part in range(2):          # 0: q, 1: k
                        g = 2 * part + hp_i // 4
                        col = (hp_i % 4) * 128
                        bcol = hp_i + 8 * part
                        if hp_i == 0 and part == 1:
                            # pair 0's last build drain gates the attention
                            # PSUM pool-open barrier with nothing to hide
                            # under. Split the k build into three groups
                            # (512/384/128 t-cols) so the LAST drain is a
                            # tiny 128-col piece: barrier exposure drops
                            # from ~810ns to ~550ns. The last piece lands
                            # in its own tile (kb) so its drain is not
                            # tile-serialized behind the 384-col drain.
                            kb = qkpool.tile([128, 128], BF16, tag="qkb",
                                             name="qkb")
                            qt["kb"] = kb
                            bias = bqk[:, bcol : bcol + 1]
                            spans = [(0, 512), (512, 896), (896, 1024)]
                            pq3 = [psum_pool.tile([128, 512], F32,
                                                  tag=psum_tag, name="pq")
                                   for _ in range(3)]

                            def kgrp(i):
                                lo, hi = spans[i]
                                for ci in range(CINT):
                                    nc.tensor.matmul(
                                        pq3[i][:, 0 : hi - lo],
                                        wqk[g][:, ci, col : col + 128],
                                        xT[:, ci, lo:hi],
                                        start=(ci == 0),
                                        stop=(ci == CINT - 1),
                                    )

                            def kfin(i):
                                lo, hi = spans[i]
                                if i == 0:
                                    nc.vector.tensor_scalar_add(
                                        qt[(1, 0)][:], pq3[0][:], bias)
                                elif i == 1:
                                    nc.vector.tensor_scalar_add(
                                        qt[(1, 1)][:, 0:384],
                                        pq3[1][:, 0:384], bias)
                                else:
                                    nc.scalar.activation(
                                        kb[:], pq3[2][:, 0:128],
                                        mybir.ActivationFunctionType.Identity,
                                        bias=bias,
                                    )

                            for i in range(3):
                                yield (lambda i=i: kgrp(i))
                                yield (lambda i=i: kfin(i))
                            continue
                        pqs = [
                            psum_pool.tile([128, 512], F32, tag=psum_tag,
                                           name="pq")
                            for _ in range(TCH)
                        ]
                        # tch-major: fully accumulate one 512-col half,
                        # then drain it (ACT for tch0, DVE for tch1) while
                        # the other half accumulates -- each drain hides
                        # under the next half's matmuls, so even pair 0's
                        # first scores never wait on a drain chain
                        for tch in range(TCH):
                            for cis in range(0, CINT, 2):
                                def step(cis=cis, tch=tch, pqs=pqs, g=g,
                                         col=col):
                                    for ci in (cis, cis + 1):
                                        nc.tensor.matmul(
                                            pqs[tch][:],
                                            wqk[g][:, ci, col : col + 128],
                                            xT[:, ci, ts(tch, 512)],
                                            start=(ci == 0),
                                            stop=(ci == CINT - 1),
                                        )
                                yield step
                            def fin_half(tch=tch, part=part, pqs=pqs,
                                         qt=qt, bcol=bcol, hp_i=hp_i):
                                bias = bqk[:, bcol : bcol + 1]
                                # ACT is the attention-span pacer (exp), so
                                # pairs 1..7 drain on DVE only. pair 0 (in
                                # the v-phase): early drain on DVE, last on
                                # ACT so the attention pool-open barrier
                                # (which waits on DVE's instruction counter)
                                # clears before the last drain
                                on_act = (hp_i == 0) and (tch == 1)
                                dst, src = qt[(part, tch)][:], pqs[tch][:]
                                if on_act:
                                    nc.scalar.activation(
                                        dst, src,
                                        mybir.ActivationFunctionType.Identity,
                                        bias=bias,
                                    )
                                else:
                                    nc.vector.tensor_scalar_add(
                                        dst, src, bias,
                                    )
                            yield fin_half

                # ---- phase 1: v(heads 0-7) = x @ W_v.T + b_v -> vvl, then
                # build(0) in the same PSUM pool (no pool-switch barrier
                # before it). v(heads 8-15) is deferred into the attention
                # windows (v_steps below). ----
                with tc.tile_pool(name="vps", bufs=8, space="PSUM") as vps:
                    for ttg in range(2):
                        pvq = [vps.tile([128, 512], F32, tag="pv", name="pv")
                               for _ in range(4)]
                        for ci in range(CINT):
                            for j in range(4):
                                tt = ttg * 4 + j
                                nc.tensor.matmul(
                                    pvq[j][:], xT[:, ci, ts(tt, 128)],
                                    wvv[0][:, ci, :],
                                    start=(ci == 0),
                                    stop=(ci == CINT - 1),
                                )
                        for j in range(4):
                            tt = ttg * 4 + j
                            nc.vector.tensor_add(
                                vvl[:, tt, :, :],
                                pvq[j][:].rearrange("p (h d) -> p h d", d=DH),
                                bv[:, 0:8, :],
                            )
                    for bs in build_steps(0, vps, "pv"):
                        bs()

                with tc.tile_pool(name="qkps", bufs=2, space="PSUM") as qkps, \
                     tc.tile_pool(name="sps", bufs=2, space="PSUM") as spsp, \
                     tc.tile_pool(name="pvp", bufs=1, space="PSUM") as pvp, \
                     tc.tile_pool(name="dnp", bufs=1, space="PSUM") as dnp:

                    def att_steps(hp_i):
                        qt = qktiles[hp_i]
                        # per-pair normalized output, [tq_p, blk, ch]-major so
                        # one DMA transpose restores the channel-major yt slab
                        ysb = att.tile([128, TT, 128], BF16, tag="ysb",
                                       name="ysb", bufs=2)
                        for tqc in range(TCH):
                            # 8 accumulation groups (2 heads x 4 tq-blocks) of
                            # [128tq, 64ch] in ONE psum bank; denominators in
                            # a second tiny tile (8 single-column groups)
                            pvt = pvp.tile([128, 8, DH], F32, tag="pv", name="pv")
                            dnt = dnp.tile([128, 8], F32, tag="dn", name="dn")
                            pend = {}

                            def scores_exp(tkt, tqc=tqc, qt=qt, pend=pend):
                                sp = spsp.tile([128, 2, 512], F32, tag="sp", name="sp")
                                for sub in range(2):
                                    po = sub * DH
                                    if tkt == TT - 1 and "kb" in qt:
                                        kst = qt["kb"][po : po + DH, :]
                                    else:
                                        kst = qt[(1, tkt // 4)][
                                            po : po + DH, ts(tkt % 4, 128)]
                                    nc.tensor.matmul(
                                        sp[:, sub, :],
                                        kst,
                                        qt[(0, tqc)][po : po + DH, :],
                                        start=True, stop=True,
                                    )
                                e = att.tile([128, 2, 512], BF16, tag="e", name="e")
                                nc.scalar.activation(e[:], sp[:], EXP, scale=SM_SCALE)
                                pend[tkt] = e

                            def pv(tkt, pvt=pvt, dnt=dnt, hp_i=hp_i, pend=pend):
                                # flipped PV: stationary exp-block, moving v.
                                # start=True zeroes the WHOLE psum bank, so
                                # only group 0's first matmul uses it (as the
                                # bank-wide zeroing); every other group
                                # accumulates with start=False (verified on
                                # hw in isolation).
                                e = pend.pop(tkt)
                                for sub in range(2):
                                    h = 2 * hp_i + sub
                                    vt = vvl if h < 8 else vvh
                                    for blk in range(4):
                                        g = sub * 4 + blk
                                        est = e[:, sub, ts(blk, 128)]
                                        first = tkt == 0 and g == 0
                                        nc.tensor.matmul(
                                            pvt[:, g, :], est,
                                            vt[:, tkt, h % 8, :],
                                            start=first, stop=(tkt == TT - 1),
                                            skip_group_check=True,
                                        )
                                        nc.tensor.matmul(
                                            dnt[:, g : g + 1], est, ones[:],
                                            start=first, stop=(tkt == TT - 1),
                                            skip_group_check=True,
                                        )

                            # one-step software pipeline: PV trails scores/exp so
                            # the in-order PE never waits on a same-step Exp
                            for tkt in range(TT):
                                def step(tkt=tkt):
                                    scores_exp(tkt)
                                    if tkt > 0:
                                        pv(tkt - 1)
                                yield step
                            def flush(tqc=tqc, hp_i=hp_i):
                                pv(TT - 1)
                            yield flush
                            def norm(tqc=tqc, pvt=pvt, dnt=dnt, ysb=ysb,
                                     hp_i=hp_i):
                                # recip frees dnt, the copy frees pvt: the
                                # next half's group-0 start matmuls gate only
                                # on these two short DVE ops, not the muls
                                rr = att.tile([128, 8], F32, tag="r", name="r",
                                              bufs=2)
                                nc.vector.reciprocal(rr[:], dnt[:])
                                yc = att.tile([128, 8, DH], BF16, tag="yc",
                                              name="yc", bufs=2)
                                nc.vector.tensor_copy(yc[:], pvt[:])
                                for sub in range(2):
                                    for blk in range(4):
                                        g = sub * 4 + blk
                                        nc.vector.tensor_scalar_mul(
                                            ysb[:, tqc * 4 + blk,
                                                sub * DH : (sub + 1) * DH],
                                            yc[:, g, :], rr[:, g : g + 1],
                                        )
                            yield norm
                        def xpose(ysb=ysb, hp_i=hp_i):
                            # [tq, ch] -> [ch, tq] on the DMA xbar (64 tiles,
                            # ~0.9us, off every compute engine)
                            nc.sync.dma_start_transpose(
                                out=yt[:, hp_i, :].rearrange(
                                    "p (b q) -> p b q", q=128),
                                in_=ysb[:],
                            )
                        yield xpose

                    def proj_tile(tt, pos_ap, otp, ci_lo=0, cch_major=False,
                                  pos_sl=None):
                        # pos_ap: callable cch -> [128, 512] PSUM AP
                        # pos_sl: callable (cch, lo, hi) -> [128, hi-lo] PSUM AP
                        ot = otp.tile([128, C], F32, tag="ot", name="ot")

                        def drain(cch):
                            nc.vector.tensor_add(
                                ot[:, ts(cch, 512)], pos_ap(cch),
                                bo[:, ts(cch, 512)],
                            )
                            nc.gpsimd.dma_start(
                                out=out_d[ts(tt, 128), ts(cch, 512)],
                                in_=ot[:, ts(cch, 512)],
                            )

                        if cch_major:
                            # close + drain the first half while the second
                            # half's matmuls still run: shortens the final tail
                            for cch in range(2):
                                for ci in range(ci_lo, CINT):
                                    nc.tensor.matmul(
                                        pos_ap(cch), yt[:, ci, ts(tt, 128)],
                                        wpa[:, ci, ts(cch, 512)],
                                        start=(ci == 0), stop=(ci == CINT - 1),
                                    )
                                if cch == 0:
                                    drain(0)
                            # final drain: one add, then two half-stores on
                            # separate DMA queues so they overlap
                            nc.vector.tensor_add(
                                ot[:, 512:1024], pos_sl(1, 0, 512),
                                bo[:, 512:1024],
                            )
                            for q in range(2):
                                sl = slice(512 + q * 256, 768 + q * 256)
                                # ACT-issued store avoids the Pool queue's
                                # larger 1,883ns DGE init for the final
                                # transfer (ACT is idle at kernel end)
                                eng = nc.scalar if q == 0 else nc.sync
                                eng.dma_start(
                                    out=out_d[ts(tt, 128), sl], in_=ot[:, sl]
                                )
                        else:
                            for ci in range(ci_lo, CINT):
                                for cch in range(2):
                                    nc.tensor.matmul(
                                        pos_ap(cch), yt[:, ci, ts(tt, 128)],
                                        wpa[:, ci, ts(cch, 512)],
                                        start=(ci == 0), stop=(ci == CINT - 1),
                                    )
                            for cch in range(2):
                                drain(cch)

                    def v_steps(hp):
                        # deferred v build for pair hp (heads 2hp, 2hp+1):
                        # two [128, 512] psum tiles in the build ring, each
                        # holding 4 tt-groups of [128, 128] (group 0's
                        # start=True doubles as the bank zeroing)
                        col = (hp % 4) * 128
                        for half in range(2):
                            vt = qkps.tile([128, 512], F32, tag="pq",
                                           name="pv2")
                            for tl in range(4):
                                def astep(vt=vt, tl=tl, tt=half * 4 + tl,
                                          col=col):
                                    for ci in range(CINT):
                                        nc.tensor.matmul(
                                            vt[:, ts(tl, 128)],
                                            xT[:, ci, ts(tt, 128)],
                                            wvv[1][:, ci, col : col + 128],
                                            start=(tl == 0 and ci == 0),
                                            stop=(ci == CINT - 1),
                                            skip_group_check=True,
                                        )
                                yield astep
                            def dstep(vt=vt, half=half, hp=hp):
                                for tl in range(4):
                                    nc.vector.tensor_add(
                                        vvh[:, half * 4 + tl,
                                            2 * (hp - 4) : 2 * (hp - 4) + 2, :],
                                        vt[:, ts(tl, 128)].rearrange(
                                            "p (h d) -> p h d", d=DH),
                                        bv[:, 2 * hp : 2 * hp + 2, :],
                                    )
                            yield dstep

                    for hp_i in range(1, NPAIR):
                        bgen = build_steps(hp_i, qkps, "pq")
                        # head start: finish the build a little before the
                        # previous pair's attention ends so the qkt drain
                        # copies are done when the next attention starts
                        for _ in range(3):
                            s = next(bgen, None)
                            if s is not None:
                                s()
                        # windows 0..3 additionally host the deferred v
                        # build for pairs 4..7 (1.5 filler steps per
                        # attention step so fillers stay interleaved)
                        if hp_i <= 4:
                            fillers = chain(bgen, v_steps(hp_i + 3))
                        else:
                            fillers = bgen
                        for i, as_ in enumerate(att_steps(hp_i - 1)):
                            f = next(fillers, None)
                            if f is not None:
                                f()
                            as_()
                            if i % 2 == 1:
                                f = next(fillers, None)
                                if f is not None:
                                    f()
                        for f in fillers:
                            f()

                    # final pair's attention has no build to hide Exp under —
                    # interleave the ci 0..6 accumulation of the first output
                    # tile into it, reusing the (idle) build PSUM banks
                    with tc.tile_pool(name="ot", bufs=3) as otp:
                        pe0 = [qkps.tile([128, 512], F32, tag="pq", name="pq")
                               for _ in range(2)]

                        def proj_early_steps():
                            # one matmul per step: spreads the extra PE work
                            # across more of the Exp-paced attention steps
                            for ci in range(CINT - 1):
                                for cch in range(2):
                                    def step(ci=ci, cch=cch):
                                        nc.tensor.matmul(
                                            pe0[cch][:], yt[:, ci, 0:128],
                                            wpa[:, ci, ts(cch, 512)],
                                            start=(ci == 0), stop=False,
                                        )
                                    yield step

                        for as_, ps in zip_longest(att_steps(NPAIR - 1),
                                                   proj_early_steps()):
                            if as_ is not None:
                                as_()
                            if ps is not None:
                                ps()
                        proj_tile(0, lambda cch: pe0[cch][:], otp,
                                  ci_lo=CINT - 1)

                        # ---- phase 3: out = y @ W_p.T + b_p ----
                        # runs inside the attention pool scope, reusing the
                        # sps ring for PSUM: no pool-switch barrier anywhere
                        for tt in range(1, TT - 1):
                            pos = spsp.tile([128, 2, 512], F32, tag="sp",
                                            name="sp")
                            proj_tile(tt, lambda cch, pos=pos: pos[:, cch, :],
                                      otp)
                        # last tile: three staggered groups (512/384/128
                        # cols) in independent PSUM tiles so the FINAL accum
                        # group is tiny -- its drain chain (add + store +
                        # fixed DMA completion constants) starts as early as
                        # possible. Separate staging tiles and three DMA
                        # queues keep the drains fully parallel.
                        tt_l = TT - 1
                        pl = [qkps.tile([128, 512], F32, tag="pq", name="pq")
                              for _ in range(3)]
                        spans = [(0, 512), (512, 800), (800, 1024)]
                        engs = [nc.gpsimd, nc.sync, nc.scalar]
                        for i in range(3):
                            lo, hi = spans[i]
                            for ci in range(CINT):
                                nc.tensor.matmul(
                                    pl[i][:, 0 : hi - lo],
                                    yt[:, ci, ts(tt_l, 128)],
                                    wpa[:, ci, lo:hi],
                                    start=(ci == 0), stop=(ci == CINT - 1),
                                )
                            oti = otp.tile([128, 512], F32, tag="otl",
                                           name="otl", bufs=3)
                            nc.vector.tensor_add(
                                oti[:, 0 : hi - lo], pl[i][:, 0 : hi - lo],
                                bo[:, lo:hi],
                            )
                            engs[i].dma_start(
                                out=out_d[ts(tt_l, 128), lo:hi],
                                in_=oti[:, 0 : hi - lo],
                            )

    nc.compile()
    return nc


_NC_CACHE = {}


def _in_maps(inputs):
    x = np.asarray(inputs["x"], dtype=np.float32)
    sid = np.asarray(inputs["subject_id"]).astype(np.int64)
    W_qkv = np.asarray(inputs["W_qkv"], dtype=np.float32)
    b_qkv = np.asarray(inputs["b_qkv"], dtype=np.float32)
    A1 = np.asarray(inputs["A1"], dtype=np.float32)
    B1 = np.asarray(inputs["B1"], dtype=np.float32)
    W_p = np.asarray(inputs["W_p"], dtype=np.float32)
    b_p = np.asarray(inputs["b_p"], dtype=np.float32)
    A2 = np.asarray(inputs["A2"], dtype=np.float32)
    B2 = np.asarray(inputs["B2"], dtype=np.float32)

    bqk = np.ascontiguousarray(b_qkv[:2048].reshape(H, 128).T)
    bv = np.ascontiguousarray(
        np.broadcast_to(b_qkv[2048:3072], (128, C)).astype(np.float32)
    )
    bo = np.ascontiguousarray(np.broadcast_to(b_p, (128, C)).astype(np.float32))

    in_maps = []
    for b in range(NCORES):
        s = int(sid[b])
        W1 = W_qkv + ALPHA_OVER_RANK * (B1[s] @ A1[s])
        Wp = W_p + ALPHA_OVER_RANK * (B2[s] @ A2[s])
        in_maps.append(
            {
                "xT": np.ascontiguousarray(x[b].T).astype(NPBF16),
                "wqkvT": np.ascontiguousarray(W1.T).astype(NPBF16),
                "wpT": np.ascontiguousarray(Wp.T).astype(NPBF16),
                "bqk": bqk,
                "bv": bv,
                "bo": bo,
            }
        )
    return in_maps


def kernel(**inputs):
    if "nc" not in _NC_CACHE:
        _NC_CACHE["nc"] = _build()
    nc = _NC_CACHE["nc"]

    res = run_bass_kernel_spmd(nc, _in_maps(inputs), core_ids=list(range(NCORES)))
    out = np.stack([r["out"] for r in res.results], axis=0)
    return out.astype(np.float32)



# revision 24
# speedup vs baseline: 1.0094x; 1.0094x over previous
"""LoRA-MHSA Trainium2 kernel.

Data-parallel over batch B=8 (one sample per NeuronCore). The per-sample LoRA
adapters are folded into the base weights on the host (exact algebra:
x@(W + (a/r)B@A).T == x@W.T + (a/r)(x@A.T)@B.T), so the device kernel is a
plain MHSA with per-core weights:
  qkv = x @ Wqkv_eff.T + b_qkv ; 16-head SDPA over T=1024, dh=64 ;
  out = y @ Wp_eff.T + b_p

All matmul operands are bf16 (full PE rate, halves DMA + SBUF); PSUM
accumulation stays fp32. Biases are added during the PSUM->SBUF drain copies
(host-replicated bias tiles), costing no extra engine time.

Layout: activations channel-major ([C, T]) so q/k head slabs feed the scores
matmul directly. Softmax needs no max-subtraction: scores are O(1) by
construction.

PV is computed FLIPPED: stationary = exp(scores) block [128tk x 128tq],
moving = v [128tk x 64ch], so each PV matmul streams only 64 columns instead
of 512 (matmul cost is out-free-size cycles; stationary loads are free).
This cuts PV from 131072 to 66560 PE cycles (~27us). The softmax
denominators come from an extra N=1 matmul per group (moving = ones column).
The flipped output is [tq, ch]-major; after the per-partition-scalar
normalize (batched reciprocal + 8 tensor_scalar muls per half), one
DMA-transpose per head pair (64 xbar tiles, ~0.9us on the otherwise idle
DMA engines) restores the channel-major yt slab the projection needs.

PSUM budget (8 banks): qk build 2 + scores 2x2 + pv accum 1 + denom 1 = 8.

Schedule (CoreSim: ~224.8us baseline; PE floor now ~191.6us):
- x streams in 8 chunks on the SP DMA queue while all weights stream on the
  Pool queue, so the v-phase GEMM starts ~2us in and is never DMA-paced
  (each DMA has a 500ns descriptor-gen floor; one chunk per consumer step
  is the optimal granularity).
- build(0) runs in the v-phase PSUM pool (no pool-switch barrier); its
  k-part accumulates as three staggered groups (512/384/128 t-cols) so the
  pool-switch barrier's one exposed drain is a tiny 128-col ACT piece.
- The q/k build of head-pair hp is interleaved step-by-step with the
  attention of pair hp-1 (3-step build head-start) so Exp (ACT) hides under
  build matmuls. Builds accumulate tch-major: each 512-col half drains
  (ACT or DVE, alternating) while the other half accumulates, and each
  pair's q/k pieces live in independent SBUF tiles (dependency tracking is
  tile-granular), so no consumer ever waits on a drain chain.
- Scores for both heads of a pair land in one 2-bank PSUM group and are
  exponentiated by a single fused ACT instruction; ys drains to SBUF
  immediately to recycle its PSUM bank.
- The last pair's attention has no build to hide Exp under: the first output
  tile's ci 0..6 projection accumulation interleaves into it using the idle
  build PSUM banks.
- The projection (weights prefetched long before) runs inside the attention
  pool scope reusing the sps PSUM ring -- no pool-switch barrier -- and the
  last tile accumulates as three staggered groups (512/320/192 cols) whose
  drains use separate staging tiles and three DMA queues, so the final
  drain chain (add + store + fixed ~2.2us DMA-completion constants) starts
  at the earliest possible instant after the last matmul.
"""

import sys
from itertools import chain, zip_longest

sys.path.insert(0, "/opt/trn_rl_repo")

import numpy as np
import ml_dtypes

import concourse.bass as bass
import concourse.tile as tile
from concourse import bacc, mybir
from concourse.bass_utils import run_bass_kernel_spmd

T = 1024
C = 1024
C3 = 3072
H = 16
DH = 64
RANK = 8
ALPHA_OVER_RANK = 1.0 / 8.0
SM_SCALE = 0.125  # 1/sqrt(dh)
NCORES = 8

F32 = mybir.dt.float32
BF16 = mybir.dt.bfloat16
EXP = mybir.ActivationFunctionType.Exp
NPBF16 = ml_dtypes.bfloat16

ts = bass.ts

TT = T // 128     # 8 t tiles
TCH = T // 512    # 2 t chunks (psum free dim)
CINT = C // 128   # 8 contraction tiles
NPAIR = H // 2    # 8 head pairs


def _build():
    nc = bacc.Bacc("TRN2", target_bir_lowering=False, debug=False)

    xT_d = nc.dram_tensor("xT", [C, T], BF16, kind="ExternalInput")
    wqkvT_d = nc.dram_tensor("wqkvT", [C, C3], BF16, kind="ExternalInput")
    wpT_d = nc.dram_tensor("wpT", [C, C], BF16, kind="ExternalInput")
    bqk_d = nc.dram_tensor("bqk", [128, H], F32, kind="ExternalInput")
    bv_d = nc.dram_tensor("bv", [128, C], F32, kind="ExternalInput")
    bo_d = nc.dram_tensor("bo", [128, C], F32, kind="ExternalInput")
    out_d = nc.dram_tensor("out", [T, C], F32, kind="ExternalOutput")

    with tile.TileContext(nc) as tc:
      with tc.tile_pool(name="res", bufs=1) as res:
        xT = res.tile([128, CINT, T], BF16, tag="xT")
        # v split: heads 0-7 built in the prologue, heads 8-15 deferred as
        # PE filler inside attention windows 0..3 (they are not needed
        # until window 4)
        vvl = res.tile([128, TT, 8, DH], BF16, tag="vvl")
        vvh = res.tile([128, TT, 8, DH], BF16, tag="vvh")
        ones = res.tile([128, 1], BF16, tag="ones")
        yt = res.tile([128, CINT, T], BF16, tag="yt")
        wpa = res.tile([128, CINT, C], BF16, tag="wpa")
        # q/k weight slabs: [g] covers 4 consecutive 128-col parts.
        # g0: q cols 0-511 (pairs 0-3), g1: q 512-1023, g2: k 1024-1535,
        # g3: k 1536-2047.
        wqk = [
            res.tile([128, CINT, 512], BF16, tag=f"wqk{g}", name=f"wqk{g}")
            for g in range(4)
        ]
        bqk = res.tile([128, H], F32, tag="bqk")
        bv = res.tile([128, H, DH], F32, tag="bv")
        bo = res.tile([128, C], F32, tag="bo")

        # ---- DMA streams: x chunks on SP, all weights on Pool ----
        # first chunk arrives in 256-col slivers so the very first stationary
        # load (xT[:, 0, 0:128]) lands ~0.6us earlier
        for s in range(4):
            nc.sync.dma_start(
                out=xT[:, 0, ts(s, 256)], in_=xT_d[0:128, ts(s, 256)]
            )
        for ci in range(1, CINT):
            nc.sync.dma_start(out=xT[:, ci, :], in_=xT_d[ts(ci, 128), :])
        nc.sync.dma_start(out=bo[:], in_=bo_d[:])
        nc.sync.dma_start(out=bqk[:], in_=bqk_d[:])

        with tc.tile_pool(name="wvp", bufs=2) as wvp:
            wvv = []
            for cch in range(2):
                w = wvp.tile([128, CINT, 512], BF16, tag="wv", name=f"wvv{cch}")
                for ci in range(CINT):
                    nc.gpsimd.dma_start(
                        out=w[:, ci, :],
                        in_=wqkvT_d[
                            ts(ci, 128), 2048 + cch * 512 : 2560 + cch * 512
                        ],
                    )
                wvv.append(w)
            nc.gpsimd.dma_start(
                out=bv[:], in_=bv_d.rearrange("p (h d) -> p h d", d=DH)
            )
            for g in range(4):
                nc.gpsimd.dma_start(
                    out=wqk[g][:],
                    in_=wqkvT_d[:, ts(g, 512)].rearrange("(n p) c -> p n c", p=128),
                )
            nc.gpsimd.dma_start(
                out=wpa[:], in_=wpT_d.rearrange("(n p) c -> p n c", p=128)
            )

            nc.vector.memset(ones[:], 1.0)

            # ---- interleaved: qk build for pair hp + attention pair hp-1 ----
            with tc.tile_pool(name="qkpool", bufs=3) as qkpool, \
                 tc.tile_pool(name="att", bufs=3) as att:

                qktiles = {}

                def build_steps(hp_i, psum_pool, psum_tag):
                    # four independent [128, 512] tiles per pair (q/k x
                    # tch-half): dependency tracking is tile-granular, so
                    # separate tiles let the first scores run as soon as
                    # the halves they actually read are drained
                    qt = {
                        (part, tch): qkpool.tile(
                            [128, 512], BF16, tag=f"qk{part}{tch}", name="qk"
                        )
                        for part in range(2) for tch in range(TCH)
                    }
                    qktiles[hp_i] = qt
                    for part in range(2):          # 0: q, 1: k
                        g = 2 * part + hp_i // 4
                        col = (hp_i % 4) * 128
                        bcol = hp_i + 8 * part
                        if hp_i == 0 and part == 1:
                            # pair 0's last build drain gates the attention
                            # PSUM pool-open barrier with nothing to hide
                            # under. Split the k build into three groups
                            # (512/384/128 t-cols) so the LAST drain is a
                            # tiny 128-col piece: barrier exposure drops
                            # from ~810ns to ~550ns. The last piece lands
                            # in its own tile (kb) so its drain is not
                            # tile-serialized behind the 384-col drain.
                            kb = qkpool.tile([128, 128], BF16, tag="qkb",
                                             name="qkb")
                            qt["kb"] = kb
                            bias = bqk[:, bcol : bcol + 1]
                            spans = [(0, 512), (512, 896), (896, 1024)]
                            pq3 = [psum_pool.tile([128, 512], F32,
                                                  tag=psum_tag, name="pq")
                                   for _ in range(3)]

                            def kgrp(i):
                                lo, hi = spans[i]
                                for ci in range(CINT):
                                    nc.tensor.matmul(
                                        pq3[i][:, 0 : hi - lo],
                                        wqk[g][:, ci, col : col + 128],
                                        xT[:, ci, lo:hi],
                                        start=(ci == 0),
                                        stop=(ci == CINT - 1),
                                    )

                            def kfin(i):
                                lo, hi = spans[i]
                                if i == 0:
                                    nc.vector.tensor_scalar_add(
                                        qt[(1, 0)][:], pq3[0][:], bias)
                                elif i == 1:
                                    nc.vector.tensor_scalar_add(
                                        qt[(1, 1)][:, 0:384],
                                        pq3[1][:, 0:384], bias)
                                else:
                                    nc.scalar.activation(
                                        kb[:], pq3[2][:, 0:128],
                                        mybir.ActivationFunctionType.Identity,
                                        bias=bias,
                                    )

                            for i in range(3):
                                yield (lambda i=i: kgrp(i))
                                yield (lambda i=i: kfin(i))
                            continue
                        pqs = [
                            psum_pool.tile([128, 512], F32, tag=psum_tag,
                                           name="pq")
                            for _ in range(TCH)
                        ]
                        # tch-major: fully accumulate one 512-col half,
                        # then drain it (ACT for tch0, DVE for tch1) while
                        # the other half accumulates -- each drain hides
                        # under the next half's matmuls, so even pair 0's
                        # first scores never wait on a drain chain
                        for tch in range(TCH):
                            for cis in range(0, CINT, 2):
                                def step(cis=cis, tch=tch, pqs=pqs, g=g,
                                         col=col):
                                    for ci in (cis, cis + 1):
                                        nc.tensor.matmul(
                                            pqs[tch][:],
                                            wqk[g][:, ci, col : col + 128],
                                            xT[:, ci, ts(tch, 512)],
                                            start=(ci == 0),
                                            stop=(ci == CINT - 1),
                                        )
                                yield step
                            def fin_half(tch=tch, part=part, pqs=pqs,
                                         qt=qt, bcol=bcol, hp_i=hp_i):
                                bias = bqk[:, bcol : bcol + 1]
                                # ACT is the attention-span pacer (exp), so
                                # pairs 1..7 drain on DVE only. pair 0 (in
                                # the v-phase): early drain on DVE, last on
                                # ACT so the attention pool-open barrier
                                # (which waits on DVE's instruction counter)
                                # clears before the last drain
                                on_act = (hp_i == 0) and (tch == 1)
                                dst, src = qt[(part, tch)][:], pqs[tch][:]
                                if on_act:
                                    nc.scalar.activation(
                                        dst, src,
                                        mybir.ActivationFunctionType.Identity,
                                        bias=bias,
                                    )
                                else:
                                    nc.vector.tensor_scalar_add(
                                        dst, src, bias,
                                    )
                            yield fin_half

                # ---- phase 1: v(heads 0-7) = x @ W_v.T + b_v -> vvl, then
                # build(0) in the same PSUM pool (no pool-switch barrier
                # before it). v(heads 8-15) is deferred into the attention
                # windows (v_steps below). ----
                with tc.tile_pool(name="vps", bufs=8, space="PSUM") as vps:
                    for ttg in range(2):
                        pvq = [vps.tile([128, 512], F32, tag="pv", name="pv")
                               for _ in range(4)]
                        for ci in range(CINT):
                            for j in range(4):
                                tt = ttg * 4 + j
                                nc.tensor.matmul(
                                    pvq[j][:], xT[:, ci, ts(tt, 128)],
                                    wvv[0][:, ci, :],
                                    start=(ci == 0),
                                    stop=(ci == CINT - 1),
                                )
                        for j in range(4):
                            tt = ttg * 4 + j
                            nc.vector.tensor_add(
                                vvl[:, tt, :, :],
                                pvq[j][:].rearrange("p (h d) -> p h d", d=DH),
                                bv[:, 0:8, :],
                            )
                    for bs in build_steps(0, vps, "pv"):
                        bs()

                with tc.tile_pool(name="qkps", bufs=2, space="PSUM") as qkps, \
                     tc.tile_pool(name="sps", bufs=2, space="PSUM") as spsp, \
                     tc.tile_pool(name="pvp", bufs=1, space="PSUM") as pvp, \
                     tc.tile_pool(name="dnp", bufs=1, space="PSUM") as dnp:

                    def att_steps(hp_i):
                        qt = qktiles[hp_i]
                        # per-pair normalized output, [tq_p, blk, ch]-major so
                        # one DMA transpose restores the channel-major yt slab
                        ysb = att.tile([128, TT, 128], BF16, tag="ysb",
                                       name="ysb", bufs=2)
                        for tqc in range(TCH):
                            # 8 accumulation groups (2 heads x 4 tq-blocks) of
                            # [128tq, 64ch] in ONE psum bank; denominators in
                            # a second tiny tile (8 single-column groups)
                            pvt = pvp.tile([128, 8, DH], F32, tag="pv", name="pv")
                            dnt = dnp.tile([128, 8], F32, tag="dn", name="dn")
                            pend = {}

                            def scores_exp(tkt, tqc=tqc, qt=qt, pend=pend):
                                sp = spsp.tile([128, 2, 512], F32, tag="sp", name="sp")
                                for sub in range(2):
                                    po = sub * DH
                                    if tkt == TT - 1 and "kb" in qt:
                                        kst = qt["kb"][po : po + DH, :]
                                    else:
                                        kst = qt[(1, tkt // 4)][
                                            po : po + DH, ts(tkt % 4, 128)]
                                    nc.tensor.matmul(
                                        sp[:, sub, :],
                                        kst,
                                        qt[(0, tqc)][po : po + DH, :],
                                        start=True, stop=True,
                                    )
                                e = att.tile([128, 2, 512], BF16, tag="e", name="e")
                                nc.scalar.activation(e[:], sp[:], EXP, scale=SM_SCALE)
                                pend[tkt] = e

                            def pv(tkt, pvt=pvt, dnt=dnt, hp_i=hp_i, pend=pend):
                                # flipped PV: stationary exp-block, moving v.
                                # start=True zeroes the WHOLE psum bank, so
                                # only group 0's first matmul uses it (as the
                                # bank-wide zeroing); every other group
                                # accumulates with start=False (verified on
                                # hw in isolation).
                                e = pend.pop(tkt)
                                for sub in range(2):
                                    h = 2 * hp_i + sub
                                    vt = vvl if h < 8 else vvh
                                    for blk in range(4):
                                        g = sub * 4 + blk
                                        est = e[:, sub, ts(blk, 128)]
                                        first = tkt == 0 and g == 0
                                        nc.tensor.matmul(
                                            pvt[:, g, :], est,
                                            vt[:, tkt, h % 8, :],
                                            start=first, stop=(tkt == TT - 1),
                                            skip_group_check=True,
                                        )
                                        nc.tensor.matmul(
                                            dnt[:, g : g + 1], est, ones[:],
                                            start=first, stop=(tkt == TT - 1),
                                            skip_group_check=True,
                                        )

                            # one-step software pipeline: PV trails scores/exp so
                            # the in-order PE never waits on a same-step Exp
                            for tkt in range(TT):
                                def step(tkt=tkt):
                                    scores_exp(tkt)
                                    if tkt > 0:
                                        pv(tkt - 1)
                                yield step
                            def flush(tqc=tqc, hp_i=hp_i):
                                pv(TT - 1)
                            yield flush
                            def norm(tqc=tqc, pvt=pvt, dnt=dnt, ysb=ysb,
                                     hp_i=hp_i):
                                # recip frees dnt, the copy frees pvt: the
                                # next half's group-0 start matmuls gate only
                                # on these two short DVE ops, not the muls
                                rr = att.tile([128, 8], F32, tag="r", name="r",
                                              bufs=2)
                                nc.vector.reciprocal(rr[:], dnt[:])
                                yc = att.tile([128, 8, DH], BF16, tag="yc",
                                              name="yc", bufs=2)
                                nc.vector.tensor_copy(yc[:], pvt[:])
                                for sub in range(2):
                                    for blk in range(4):
                                        g = sub * 4 + blk
                                        nc.vector.tensor_scalar_mul(
                                            ysb[:, tqc * 4 + blk,
                                                sub * DH : (sub + 1) * DH],
                                            yc[:, g, :], rr[:, g : g + 1],
                                        )
                            yield norm
                        def xpose(ysb=ysb, hp_i=hp_i):
                            # [tq, ch] -> [ch, tq] on the DMA xbar (64 tiles,
                            # ~0.9us, off every compute engine)
                            nc.sync.dma_start_transpose(
                                out=yt[:, hp_i, :].rearrange(
                                    "p (b q) -> p b q", q=128),
                                in_=ysb[:],
                            )
                        yield xpose

                    def proj_tile(tt, pos_ap, otp, ci_lo=0, cch_major=False,
                                  pos_sl=None, bias_ap=None, start_ci=0):
                        # pos_ap: callable cch -> [128, 512] PSUM AP
                        # pos_sl: callable (cch, lo, hi) -> [128, hi-lo] PSUM AP
                        # bias_ap: callable cch -> [128, 512] addend
                        # (defaults to the bias slab)
                        if bias_ap is None:
                            bias_ap = lambda cch: bo[:, ts(cch, 512)]
                        ot = otp.tile([128, C], F32, tag="ot", name="ot")

                        def drain(cch):
                            nc.vector.tensor_add(
                                ot[:, ts(cch, 512)], pos_ap(cch),
                                bias_ap(cch),
                            )
                            nc.gpsimd.dma_start(
                                out=out_d[ts(tt, 128), ts(cch, 512)],
                                in_=ot[:, ts(cch, 512)],
                            )

                        if cch_major:
                            # close + drain the first half while the second
                            # half's matmuls still run: shortens the final tail
                            for cch in range(2):
                                for ci in range(ci_lo, CINT):
                                    nc.tensor.matmul(
                                        pos_ap(cch), yt[:, ci, ts(tt, 128)],
                                        wpa[:, ci, ts(cch, 512)],
                                        start=(ci == 0), stop=(ci == CINT - 1),
                                    )
                                if cch == 0:
                                    drain(0)
                            # final drain: one add, then two half-stores on
                            # separate DMA queues so they overlap
                            nc.vector.tensor_add(
                                ot[:, 512:1024], pos_sl(1, 0, 512),
                                bo[:, 512:1024],
                            )
                            for q in range(2):
                                sl = slice(512 + q * 256, 768 + q * 256)
                                # ACT-issued store avoids the Pool queue's
                                # larger 1,883ns DGE init for the final
                                # transfer (ACT is idle at kernel end)
                                eng = nc.scalar if q == 0 else nc.sync
                                eng.dma_start(
                                    out=out_d[ts(tt, 128), sl], in_=ot[:, sl]
                                )
                        else:
                            for ci in range(ci_lo, CINT):
                                for cch in range(2):
                                    nc.tensor.matmul(
                                        pos_ap(cch), yt[:, ci, ts(tt, 128)],
                                        wpa[:, ci, ts(cch, 512)],
                                        start=(ci == start_ci),
                                        stop=(ci == CINT - 1),
                                    )
                            for cch in range(2):
                                drain(cch)

                    def v_steps(hp):
                        # deferred v build for pair hp (heads 2hp, 2hp+1):
                        # two [128, 512] psum tiles in the build ring, each
                        # holding 4 tt-groups of [128, 128] (group 0's
                        # start=True doubles as the bank zeroing)
                        col = (hp % 4) * 128
                        for half in range(2):
                            vt = qkps.tile([128, 512], F32, tag="pq",
                                           name="pv2")
                            for tl in range(4):
                                def astep(vt=vt, tl=tl, tt=half * 4 + tl,
                                          col=col):
                                    for ci in range(CINT):
                                        nc.tensor.matmul(
                                            vt[:, ts(tl, 128)],
                                            xT[:, ci, ts(tt, 128)],
                                            wvv[1][:, ci, col : col + 128],
                                            start=(tl == 0 and ci == 0),
                                            stop=(ci == CINT - 1),
                                            skip_group_check=True,
                                        )
                                yield astep
                            def dstep(vt=vt, half=half, hp=hp):
                                for tl in range(4):
                                    nc.vector.tensor_add(
                                        vvh[:, half * 4 + tl,
                                            2 * (hp - 4) : 2 * (hp - 4) + 2, :],
                                        vt[:, ts(tl, 128)].rearrange(
                                            "p (h d) -> p h d", d=DH),
                                        bv[:, 2 * hp : 2 * hp + 2, :],
                                    )
                            yield dstep

                    for hp_i in range(1, NPAIR):
                        bgen = build_steps(hp_i, qkps, "pq")
                        # head start: finish the build a little before the
                        # previous pair's attention ends so the qkt drain
                        # copies are done when the next attention starts
                        for _ in range(3):
                            s = next(bgen, None)
                            if s is not None:
                                s()
                        # windows 0..3 additionally host the deferred v
                        # build for pairs 4..7 (1.5 filler steps per
                        # attention step so fillers stay interleaved)
                        if hp_i <= 4:
                            fillers = chain(bgen, v_steps(hp_i + 3))
                        else:
                            fillers = bgen
                        for i, as_ in enumerate(att_steps(hp_i - 1)):
                            f = next(fillers, None)
                            if f is not None:
                                f()
                            as_()
                            # only the v-hosting windows need the denser
                            # 1.5x pacing; build-only windows stay 1:1 so
                            # the build spreads across the whole window
                            if hp_i <= 4 and i % 2 == 1:
                                f = next(fillers, None)
                                if f is not None:
                                    f()
                        for f in fillers:
                            f()

                    # final pair's attention has no build to hide Exp under —
                    # interleave the ci 0..6 accumulation of output tiles 0
                    # AND 1 into it: tile 0 fills the two build banks, is
                    # drained early (bias folded in) to SBUF partials, and
                    # the banks are reused for tile 1, which stays resident
                    # for the tail
                    with tc.tile_pool(name="ot", bufs=3) as otp:
                        pe0 = [qkps.tile([128, 512], F32, tag="pq", name="pq")
                               for _ in range(2)]
                        sbp = [att.tile([128, 512], F32, tag=f"sbp{c}",
                                        name=f"sbp{c}", bufs=1)
                               for c in range(2)]

                        def proj_early_steps():
                            # one matmul per step: spreads the extra PE work
                            # across more of the Exp-paced attention steps
                            for cch in range(2):
                                for ci in range(CINT - 1):
                                    def step(ci=ci, cch=cch):
                                        nc.tensor.matmul(
                                            pe0[cch][:], yt[:, ci, 0:128],
                                            wpa[:, ci, ts(cch, 512)],
                                            start=(ci == 0),
                                            stop=(ci == CINT - 2),
                                            skip_group_check=True,
                                        )
                                    yield step
                            for cch in range(2):
                                def dstep(cch=cch):
                                    nc.vector.tensor_add(
                                        sbp[cch][:], pe0[cch][:],
                                        bo[:, ts(cch, 512)],
                                    )
                                yield dstep
                            for cch in range(2):
                                for ci in range(CINT - 1):
                                    def step2(ci=ci, cch=cch):
                                        nc.tensor.matmul(
                                            pe0[cch][:], yt[:, ci, 128:256],
                                            wpa[:, ci, ts(cch, 512)],
                                            start=(ci == 0), stop=False,
                                            skip_group_check=True,
                                        )
                                    yield step2

                        for as_, ps in zip_longest(att_steps(NPAIR - 1),
                                                   proj_early_steps()):
                            if as_ is not None:
                                as_()
                            if ps is not None:
                                ps()

                        # ---- phase 3: out = y @ W_p.T + b_p ----
                        # runs inside the attention pool scope, reusing the
                        # sps ring for PSUM: no pool-switch barrier anywhere.
                        # tile 2 first (its ci7 gives the pair-7 transpose
                        # time to land), then tiles 0/1 whose early partials
                        # only need their ci7 pass.
                        pos2 = spsp.tile([128, 2, 512], F32, tag="sp",
                                         name="sp")
                        proj_tile(2, lambda cch: pos2[:, cch, :], otp)
                        pos0 = spsp.tile([128, 2, 512], F32, tag="sp",
                                         name="sp")
                        proj_tile(0, lambda cch: pos0[:, cch, :], otp,
                                  ci_lo=CINT - 1, start_ci=CINT - 1,
                                  bias_ap=lambda cch: sbp[cch][:])
                        proj_tile(1, lambda cch: pe0[cch][:], otp,
                                  ci_lo=CINT - 1)
                        for tt in range(3, TT - 1):
                            pos = spsp.tile([128, 2, 512], F32, tag="sp",
                                            name="sp")
                            proj_tile(tt, lambda cch, pos=pos: pos[:, cch, :],
                                      otp)
                        # last tile: three staggered groups (512/384/128
                        # cols) in independent PSUM tiles so the FINAL accum
                        # group is tiny -- its drain chain (add + store +
                        # fixed DMA completion constants) starts as early as
                        # possible. Separate staging tiles and three DMA
                        # queues keep the drains fully parallel.
                        tt_l = TT - 1
                        pl = [qkps.tile([128, 512], F32, tag="pq", name="pq")
                              for _ in range(3)]
                        spans = [(0, 512), (512, 800), (800, 1024)]
                        engs = [nc.gpsimd, nc.sync, nc.scalar]
                        for i in range(3):
                            lo, hi = spans[i]
                            for ci in range(CINT):
                                nc.tensor.matmul(
                                    pl[i][:, 0 : hi - lo],
                                    yt[:, ci, ts(tt_l, 128)],
                                    wpa[:, ci, lo:hi],
                                    start=(ci == 0), stop=(ci == CINT - 1),
                                )
                            oti = otp.tile([128, 512], F32, tag="otl",
                                           name="otl", bufs=3)
                            nc.vector.tensor_add(
                                oti[:, 0 : hi - lo], pl[i][:, 0 : hi - lo],
                                bo[:, lo:hi],
                            )
                            engs[i].dma_start(
                                out=out_d[ts(tt_l, 128), lo:hi],
                                in_=oti[:, 0 : hi - lo],
                            )

    nc.compile()
    return nc


_NC_CACHE = {}


def _in_maps(inputs):
    x = np.asarray(inputs["x"], dtype=np.float32)
    sid = np.asarray(inputs["subject_id"]).astype(np.int64)
    W_qkv = np.asarray(inputs["W_qkv"], dtype=np.float32)
    b_qkv = np.asarray(inputs["b_qkv"], dtype=np.float32)
    A1 = np.asarray(inputs["A1"], dtype=np.float32)
    B1 = np.asarray(inputs["B1"], dtype=np.float32)
    W_p = np.asarray(inputs["W_p"], dtype=np.float32)
    b_p = np.asarray(inputs["b_p"], dtype=np.float32)
    A2 = np.asarray(inputs["A2"], dtype=np.float32)
    B2 = np.asarray(inputs["B2"], dtype=np.float32)

    bqk = np.ascontiguousarray(b_qkv[:2048].reshape(H, 128).T)
    bv = np.ascontiguousarray(
        np.broadcast_to(b_qkv[2048:3072], (128, C)).astype(np.float32)
    )
    bo = np.ascontiguousarray(np.broadcast_to(b_p, (128, C)).astype(np.float32))

    in_maps = []
    for b in range(NCORES):
        s = int(sid[b])
        W1 = W_qkv + ALPHA_OVER_RANK * (B1[s] @ A1[s])
        Wp = W_p + ALPHA_OVER_RANK * (B2[s] @ A2[s])
        in_maps.append(
            {
                "xT": np.ascontiguousarray(x[b].T).astype(NPBF16),
                "wqkvT": np.ascontiguousarray(W1.T).astype(NPBF16),
                "wpT": np.ascontiguousarray(Wp.T).astype(NPBF16),
                "bqk": bqk,
                "bv": bv,
                "bo": bo,
            }
        )
    return in_maps


def kernel(**inputs):
    if "nc" not in _NC_CACHE:
        _NC_CACHE["nc"] = _build()
    nc = _NC_CACHE["nc"]

    res = run_bass_kernel_spmd(nc, _in_maps(inputs), core_ids=list(range(NCORES)))
    out = np.stack([r["out"] for r in res.results], axis=0)
    return out.astype(np.float32)



# revision 26
# speedup vs baseline: 1.0185x; 1.0090x over previous
"""LoRA-MHSA Trainium2 kernel.

Data-parallel over batch B=8 (one sample per NeuronCore). The per-sample LoRA
adapters are folded into the base weights on the host (exact algebra:
x@(W + (a/r)B@A).T == x@W.T + (a/r)(x@A.T)@B.T), so the device kernel is a
plain MHSA with per-core weights:
  qkv = x @ Wqkv_eff.T + b_qkv ; 16-head SDPA over T=1024, dh=64 ;
  out = y @ Wp_eff.T + b_p

All matmul operands are bf16 (full PE rate, halves DMA + SBUF); PSUM
accumulation stays fp32. Biases are added during the PSUM->SBUF drain copies
(host-replicated bias tiles), costing no extra engine time.

Layout: activations channel-major ([C, T]) so q/k head slabs feed the scores
matmul directly. Softmax needs no max-subtraction: scores are O(1) by
construction.

PV is computed FLIPPED: stationary = exp(scores) block [128tk x 128tq],
moving = v [128tk x 64ch], so each PV matmul streams only 64 columns instead
of 512 (matmul cost is out-free-size cycles; stationary loads are free).
This cuts PV from 131072 to 66560 PE cycles (~27us). The softmax
denominators come from an extra N=1 matmul per group (moving = ones column).
The flipped output is [tq, ch]-major; after the per-partition-scalar
normalize (batched reciprocal + 8 tensor_scalar muls per half), one
DMA-transpose per head pair (64 xbar tiles, ~0.9us on the otherwise idle
DMA engines) restores the channel-major yt slab the projection needs.

PSUM budget (8 banks): qk build 2 + scores 2x2 + pv accum 1 + denom 1 = 8.

Schedule (CoreSim: ~224.8us baseline; PE floor now ~191.6us):
- x streams in 8 chunks on the SP DMA queue while all weights stream on the
  Pool queue, so the v-phase GEMM starts ~2us in and is never DMA-paced
  (each DMA has a 500ns descriptor-gen floor; one chunk per consumer step
  is the optimal granularity).
- build(0) runs in the v-phase PSUM pool (no pool-switch barrier); its
  k-part accumulates as three staggered groups (512/384/128 t-cols) so the
  pool-switch barrier's one exposed drain is a tiny 128-col ACT piece.
- The q/k build of head-pair hp is interleaved step-by-step with the
  attention of pair hp-1 (3-step build head-start) so Exp (ACT) hides under
  build matmuls. Builds accumulate tch-major: each 512-col half drains
  (ACT or DVE, alternating) while the other half accumulates, and each
  pair's q/k pieces live in independent SBUF tiles (dependency tracking is
  tile-granular), so no consumer ever waits on a drain chain.
- Scores for both heads of a pair land in one 2-bank PSUM group and are
  exponentiated by a single fused ACT instruction; ys drains to SBUF
  immediately to recycle its PSUM bank.
- The last pair's attention has no build to hide Exp under: the first output
  tile's ci 0..6 projection accumulation interleaves into it using the idle
  build PSUM banks.
- The projection (weights prefetched long before) runs inside the attention
  pool scope reusing the sps PSUM ring -- no pool-switch barrier -- and the
  last tile accumulates as three staggered groups (512/320/192 cols) whose
  drains use separate staging tiles and three DMA queues, so the final
  drain chain (add + store + fixed ~2.2us DMA-completion constants) starts
  at the earliest possible instant after the last matmul.
"""

import sys
from itertools import chain, zip_longest

sys.path.insert(0, "/opt/trn_rl_repo")

import numpy as np
import ml_dtypes

import concourse.bass as bass
import concourse.tile as tile
from concourse import bacc, mybir
from concourse.bass_utils import run_bass_kernel_spmd

T = 1024
C = 1024
C3 = 3072
H = 16
DH = 64
RANK = 8
ALPHA_OVER_RANK = 1.0 / 8.0
SM_SCALE = 0.125  # 1/sqrt(dh)
NCORES = 8

F32 = mybir.dt.float32
BF16 = mybir.dt.bfloat16
EXP = mybir.ActivationFunctionType.Exp
NPBF16 = ml_dtypes.bfloat16

ts = bass.ts

TT = T // 128     # 8 t tiles
TCH = T // 512    # 2 t chunks (psum free dim)
CINT = C // 128   # 8 contraction tiles
NPAIR = H // 2    # 8 head pairs


def _build():
    nc = bacc.Bacc("TRN2", target_bir_lowering=False, debug=False)

    xT_d = nc.dram_tensor("xT", [C, T], BF16, kind="ExternalInput")
    wqkvT_d = nc.dram_tensor("wqkvT", [C, C3], BF16, kind="ExternalInput")
    wpT_d = nc.dram_tensor("wpT", [C, C], BF16, kind="ExternalInput")
    bqk_d = nc.dram_tensor("bqk", [128, H], F32, kind="ExternalInput")
    bv_d = nc.dram_tensor("bv", [128, C], F32, kind="ExternalInput")
    bo_d = nc.dram_tensor("bo", [128, C], F32, kind="ExternalInput")
    out_d = nc.dram_tensor("out", [T, C], F32, kind="ExternalOutput")

    with tile.TileContext(nc) as tc:
      with tc.tile_pool(name="res", bufs=1) as res:
        xT = res.tile([128, CINT, T], BF16, tag="xT")
        # v split: heads 0-7 built in the prologue, heads 8-15 deferred as
        # PE filler inside attention windows 0..3 (they are not needed
        # until window 4)
        vvl = res.tile([128, TT, 8, DH], BF16, tag="vvl")
        vvh = res.tile([128, TT, 8, DH], BF16, tag="vvh")
        ones = res.tile([128, 1], BF16, tag="ones")
        yt = res.tile([128, CINT, T], BF16, tag="yt")
        wpa = res.tile([128, CINT, C], BF16, tag="wpa")
        # q/k weight slabs: [g] covers 4 consecutive 128-col parts.
        # g0: q cols 0-511 (pairs 0-3), g1: q 512-1023, g2: k 1024-1535,
        # g3: k 1536-2047.
        wqk = [
            res.tile([128, CINT, 512], BF16, tag=f"wqk{g}", name=f"wqk{g}")
            for g in range(4)
        ]
        bqk = res.tile([128, H], F32, tag="bqk")
        bv = res.tile([128, H, DH], F32, tag="bv")
        bo = res.tile([128, C], F32, tag="bo")

        # ---- DMA streams: x chunks on SP, all weights on Pool ----
        # first chunk arrives in 256-col slivers so the very first stationary
        # load (xT[:, 0, 0:128]) lands ~0.6us earlier
        for s in range(4):
            nc.sync.dma_start(
                out=xT[:, 0, ts(s, 256)], in_=xT_d[0:128, ts(s, 256)]
            )
        for ci in range(1, CINT):
            nc.sync.dma_start(out=xT[:, ci, :], in_=xT_d[ts(ci, 128), :])
        nc.sync.dma_start(out=bo[:], in_=bo_d[:])
        nc.sync.dma_start(out=bqk[:], in_=bqk_d[:])

        with tc.tile_pool(name="wvp", bufs=2) as wvp:
            wvv = []
            for cch in range(2):
                w = wvp.tile([128, CINT, 512], BF16, tag="wv", name=f"wvv{cch}")
                for ci in range(CINT):
                    nc.gpsimd.dma_start(
                        out=w[:, ci, :],
                        in_=wqkvT_d[
                            ts(ci, 128), 2048 + cch * 512 : 2560 + cch * 512
                        ],
                    )
                wvv.append(w)
            nc.gpsimd.dma_start(
                out=bv[:], in_=bv_d.rearrange("p (h d) -> p h d", d=DH)
            )
            for g in range(4):
                nc.gpsimd.dma_start(
                    out=wqk[g][:],
                    in_=wqkvT_d[:, ts(g, 512)].rearrange("(n p) c -> p n c", p=128),
                )
            nc.gpsimd.dma_start(
                out=wpa[:], in_=wpT_d.rearrange("(n p) c -> p n c", p=128)
            )

            nc.vector.memset(ones[:], 1.0)

            # ---- interleaved: qk build for pair hp + attention pair hp-1 ----
            with tc.tile_pool(name="qkpool", bufs=3) as qkpool, \
                 tc.tile_pool(name="att", bufs=3) as att:

                qktiles = {}

                def build_steps(hp_i, psum_pool, psum_tag):
                    # four independent [128, 512] tiles per pair (q/k x
                    # tch-half): dependency tracking is tile-granular, so
                    # separate tiles let the first scores run as soon as
                    # the halves they actually read are drained
                    qt = {
                        (part, tch): qkpool.tile(
                            [128, 512], BF16, tag=f"qk{part}{tch}", name="qk"
                        )
                        for part in range(2) for tch in range(TCH)
                    }
                    qktiles[hp_i] = qt
                    for part in range(2):          # 0: q, 1: k
                        g = 2 * part + hp_i // 4
                        col = (hp_i % 4) * 128
                        bcol = hp_i + 8 * part
                        if hp_i == 0 and part == 1:
                            # pair 0's last build drain gates the attention
                            # PSUM pool-open barrier with nothing to hide
                            # under. Split the k build into three groups
                            # (512/384/128 t-cols) so the LAST drain is a
                            # tiny 128-col piece: barrier exposure drops
                            # from ~810ns to ~550ns. The last piece lands
                            # in its own tile (kb) so its drain is not
                            # tile-serialized behind the 384-col drain.
                            kb = qkpool.tile([128, 128], BF16, tag="qkb",
                                             name="qkb")
                            qt["kb"] = kb
                            bias = bqk[:, bcol : bcol + 1]
                            spans = [(0, 512), (512, 896), (896, 1024)]
                            pq3 = [psum_pool.tile([128, 512], F32,
                                                  tag=psum_tag, name="pq")
                                   for _ in range(3)]

                            def kgrp(i):
                                lo, hi = spans[i]
                                for ci in range(CINT):
                                    nc.tensor.matmul(
                                        pq3[i][:, 0 : hi - lo],
                                        wqk[g][:, ci, col : col + 128],
                                        xT[:, ci, lo:hi],
                                        start=(ci == 0),
                                        stop=(ci == CINT - 1),
                                    )

                            def kfin(i):
                                lo, hi = spans[i]
                                if i == 0:
                                    nc.vector.tensor_scalar_add(
                                        qt[(1, 0)][:], pq3[0][:], bias)
                                elif i == 1:
                                    nc.vector.tensor_scalar_add(
                                        qt[(1, 1)][:, 0:384],
                                        pq3[1][:, 0:384], bias)
                                else:
                                    nc.scalar.activation(
                                        kb[:], pq3[2][:, 0:128],
                                        mybir.ActivationFunctionType.Identity,
                                        bias=bias,
                                    )

                            for i in range(3):
                                yield (lambda i=i: kgrp(i))
                                yield (lambda i=i: kfin(i))
                            continue
                        pqs = [
                            psum_pool.tile([128, 512], F32, tag=psum_tag,
                                           name="pq")
                            for _ in range(TCH)
                        ]
                        # tch-major: fully accumulate one 512-col half,
                        # then drain it (ACT for tch0, DVE for tch1) while
                        # the other half accumulates -- each drain hides
                        # under the next half's matmuls, so even pair 0's
                        # first scores never wait on a drain chain
                        for tch in range(TCH):
                            for cis in range(0, CINT, 2):
                                def step(cis=cis, tch=tch, pqs=pqs, g=g,
                                         col=col):
                                    for ci in (cis, cis + 1):
                                        nc.tensor.matmul(
                                            pqs[tch][:],
                                            wqk[g][:, ci, col : col + 128],
                                            xT[:, ci, ts(tch, 512)],
                                            start=(ci == 0),
                                            stop=(ci == CINT - 1),
                                        )
                                yield step
                            def fin_half(tch=tch, part=part, pqs=pqs,
                                         qt=qt, bcol=bcol, hp_i=hp_i):
                                bias = bqk[:, bcol : bcol + 1]
                                # ACT is the attention-span pacer (exp), so
                                # pairs 1..7 drain on DVE only. pair 0 (in
                                # the v-phase): early drain on DVE, last on
                                # ACT so the attention pool-open barrier
                                # (which waits on DVE's instruction counter)
                                # clears before the last drain
                                on_act = (hp_i == 0) and (tch == 1)
                                dst, src = qt[(part, tch)][:], pqs[tch][:]
                                if on_act:
                                    nc.scalar.activation(
                                        dst, src,
                                        mybir.ActivationFunctionType.Identity,
                                        bias=bias,
                                    )
                                else:
                                    nc.vector.tensor_scalar_add(
                                        dst, src, bias,
                                    )
                            yield fin_half

                # ---- phase 1: v(heads 0-7) = x @ W_v.T + b_v -> vvl, then
                # build(0) in the same PSUM pool (no pool-switch barrier
                # before it). v(heads 8-15) is deferred into the attention
                # windows (v_steps below). ----
                with tc.tile_pool(name="vps", bufs=8, space="PSUM") as vps:
                    for ttg in range(2):
                        pvq = [vps.tile([128, 512], F32, tag="pv", name="pv")
                               for _ in range(4)]
                        for ci in range(CINT):
                            for j in range(4):
                                tt = ttg * 4 + j
                                nc.tensor.matmul(
                                    pvq[j][:], xT[:, ci, ts(tt, 128)],
                                    wvv[0][:, ci, :],
                                    start=(ci == 0),
                                    stop=(ci == CINT - 1),
                                )
                        for j in range(4):
                            tt = ttg * 4 + j
                            nc.vector.tensor_add(
                                vvl[:, tt, :, :],
                                pvq[j][:].rearrange("p (h d) -> p h d", d=DH),
                                bv[:, 0:8, :],
                            )
                    for bs in build_steps(0, vps, "pv"):
                        bs()

                with tc.tile_pool(name="qkps", bufs=2, space="PSUM") as qkps, \
                     tc.tile_pool(name="sps", bufs=2, space="PSUM") as spsp, \
                     tc.tile_pool(name="pvp", bufs=1, space="PSUM") as pvp, \
                     tc.tile_pool(name="dnp", bufs=1, space="PSUM") as dnp:

                    def att_steps(hp_i):
                        qt = qktiles[hp_i]
                        # per-pair normalized output, [tq_p, blk, ch]-major so
                        # one DMA transpose restores the channel-major yt slab
                        ysb = att.tile([128, TT, 128], BF16, tag="ysb",
                                       name="ysb", bufs=2)
                        for tqc in range(TCH):
                            # 8 accumulation groups (2 heads x 4 tq-blocks) of
                            # [128tq, 64ch] in ONE psum bank; denominators in
                            # a second tiny tile (8 single-column groups)
                            pvt = pvp.tile([128, 8, DH], F32, tag="pv", name="pv")
                            dnt = dnp.tile([128, 8], F32, tag="dn", name="dn")
                            pend = {}

                            def scores_exp(tkt, tqc=tqc, qt=qt, pend=pend):
                                sp = spsp.tile([128, 2, 512], F32, tag="sp", name="sp")
                                for sub in range(2):
                                    po = sub * DH
                                    if tkt == TT - 1 and "kb" in qt:
                                        kst = qt["kb"][po : po + DH, :]
                                    else:
                                        kst = qt[(1, tkt // 4)][
                                            po : po + DH, ts(tkt % 4, 128)]
                                    nc.tensor.matmul(
                                        sp[:, sub, :],
                                        kst,
                                        qt[(0, tqc)][po : po + DH, :],
                                        start=True, stop=True,
                                    )
                                e = att.tile([128, 2, 512], BF16, tag="e", name="e")
                                nc.scalar.activation(e[:], sp[:], EXP, scale=SM_SCALE)
                                pend[tkt] = e

                            def pv(tkt, pvt=pvt, dnt=dnt, hp_i=hp_i, pend=pend):
                                # flipped PV: stationary exp-block, moving v.
                                # start=True zeroes the WHOLE psum bank, so
                                # only group 0's first matmul uses it (as the
                                # bank-wide zeroing); every other group
                                # accumulates with start=False (verified on
                                # hw in isolation).
                                e = pend.pop(tkt)
                                for sub in range(2):
                                    h = 2 * hp_i + sub
                                    vt = vvl if h < 8 else vvh
                                    for blk in range(4):
                                        g = sub * 4 + blk
                                        est = e[:, sub, ts(blk, 128)]
                                        first = tkt == 0 and g == 0
                                        nc.tensor.matmul(
                                            pvt[:, g, :], est,
                                            vt[:, tkt, h % 8, :],
                                            start=first, stop=(tkt == TT - 1),
                                            skip_group_check=True,
                                        )
                                        nc.tensor.matmul(
                                            dnt[:, g : g + 1], est, ones[:],
                                            start=first, stop=(tkt == TT - 1),
                                            skip_group_check=True,
                                        )

                            # one-step software pipeline: PV trails scores/exp so
                            # the in-order PE never waits on a same-step Exp
                            for tkt in range(TT):
                                def step(tkt=tkt):
                                    scores_exp(tkt)
                                    if tkt > 0:
                                        pv(tkt - 1)
                                yield step
                            def flush(tqc=tqc, hp_i=hp_i):
                                pv(TT - 1)
                            yield flush
                            def norm(tqc=tqc, pvt=pvt, dnt=dnt, ysb=ysb,
                                     hp_i=hp_i):
                                # recip frees dnt, the copy frees pvt: the
                                # next half's group-0 start matmuls gate only
                                # on these two short DVE ops, not the muls
                                rr = att.tile([128, 8], F32, tag="r", name="r",
                                              bufs=2)
                                nc.vector.reciprocal(rr[:], dnt[:])
                                yc = att.tile([128, 8, DH], BF16, tag="yc",
                                              name="yc", bufs=2)
                                nc.vector.tensor_copy(yc[:], pvt[:])
                                for sub in range(2):
                                    for blk in range(4):
                                        g = sub * 4 + blk
                                        nc.vector.tensor_scalar_mul(
                                            ysb[:, tqc * 4 + blk,
                                                sub * DH : (sub + 1) * DH],
                                            yc[:, g, :], rr[:, g : g + 1],
                                        )
                            yield norm
                            def xpose(tqc=tqc, ysb=ysb, hp_i=hp_i):
                                # [tq, ch] -> [ch, tq] on the DMA xbar (32
                                # tiles, ~0.45us, off every compute engine).
                                # Per-half so the last pair's first half of
                                # yt is ready long before the projection
                                # tail needs it.
                                nc.sync.dma_start_transpose(
                                    out=yt[:, hp_i, ts(tqc, 512)].rearrange(
                                        "p (b q) -> p b q", q=128),
                                    in_=ysb[:, tqc * 4 : (tqc + 1) * 4, :],
                                )
                            yield xpose

                    def proj_tile(tt, pos_ap, otp, ci_lo=0, cch_major=False,
                                  pos_sl=None, bias_ap=None, start_ci=0):
                        # pos_ap: callable cch -> [128, 512] PSUM AP
                        # pos_sl: callable (cch, lo, hi) -> [128, hi-lo] PSUM AP
                        # bias_ap: callable cch -> [128, 512] addend
                        # (defaults to the bias slab)
                        if bias_ap is None:
                            bias_ap = lambda cch: bo[:, ts(cch, 512)]
                        ot = otp.tile([128, C], F32, tag="ot", name="ot")

                        def drain(cch):
                            nc.vector.tensor_add(
                                ot[:, ts(cch, 512)], pos_ap(cch),
                                bias_ap(cch),
                            )
                            nc.gpsimd.dma_start(
                                out=out_d[ts(tt, 128), ts(cch, 512)],
                                in_=ot[:, ts(cch, 512)],
                            )

                        if cch_major:
                            # close + drain the first half while the second
                            # half's matmuls still run: shortens the final tail
                            for cch in range(2):
                                for ci in range(ci_lo, CINT):
                                    nc.tensor.matmul(
                                        pos_ap(cch), yt[:, ci, ts(tt, 128)],
                                        wpa[:, ci, ts(cch, 512)],
                                        start=(ci == 0), stop=(ci == CINT - 1),
                                    )
                                if cch == 0:
                                    drain(0)
                            # final drain: one add, then two half-stores on
                            # separate DMA queues so they overlap
                            nc.vector.tensor_add(
                                ot[:, 512:1024], pos_sl(1, 0, 512),
                                bo[:, 512:1024],
                            )
                            for q in range(2):
                                sl = slice(512 + q * 256, 768 + q * 256)
                                # ACT-issued store avoids the Pool queue's
                                # larger 1,883ns DGE init for the final
                                # transfer (ACT is idle at kernel end)
                                eng = nc.scalar if q == 0 else nc.sync
                                eng.dma_start(
                                    out=out_d[ts(tt, 128), sl], in_=ot[:, sl]
                                )
                        else:
                            for ci in range(ci_lo, CINT):
                                for cch in range(2):
                                    nc.tensor.matmul(
                                        pos_ap(cch), yt[:, ci, ts(tt, 128)],
                                        wpa[:, ci, ts(cch, 512)],
                                        start=(ci == start_ci),
                                        stop=(ci == CINT - 1),
                                    )
                            for cch in range(2):
                                drain(cch)

                    def v_steps(hp):
                        # deferred v build for pair hp (heads 2hp, 2hp+1):
                        # two [128, 512] psum tiles in the build ring, each
                        # holding 4 tt-groups of [128, 128] (group 0's
                        # start=True doubles as the bank zeroing)
                        col = (hp % 4) * 128
                        for half in range(2):
                            vt = qkps.tile([128, 512], F32, tag="pq",
                                           name="pv2")
                            for tl in range(4):
                                def astep(vt=vt, tl=tl, tt=half * 4 + tl,
                                          col=col):
                                    for ci in range(CINT):
                                        nc.tensor.matmul(
                                            vt[:, ts(tl, 128)],
                                            xT[:, ci, ts(tt, 128)],
                                            wvv[1][:, ci, col : col + 128],
                                            start=(tl == 0 and ci == 0),
                                            stop=(ci == CINT - 1),
                                            skip_group_check=True,
                                        )
                                yield astep
                            def dstep(vt=vt, half=half, hp=hp):
                                for tl in range(4):
                                    nc.vector.tensor_add(
                                        vvh[:, half * 4 + tl,
                                            2 * (hp - 4) : 2 * (hp - 4) + 2, :],
                                        vt[:, ts(tl, 128)].rearrange(
                                            "p (h d) -> p h d", d=DH),
                                        bv[:, 2 * hp : 2 * hp + 2, :],
                                    )
                            yield dstep

                    for hp_i in range(1, NPAIR):
                        bgen = build_steps(hp_i, qkps, "pq")
                        # head start: finish the build a little before the
                        # previous pair's attention ends so the qkt drain
                        # copies are done when the next attention starts
                        for _ in range(3):
                            s = next(bgen, None)
                            if s is not None:
                                s()
                        # windows 0..3 additionally host the deferred v
                        # build for pairs 4..7 (1.5 filler steps per
                        # attention step so fillers stay interleaved)
                        if hp_i <= 4:
                            fillers = chain(bgen, v_steps(hp_i + 3))
                        else:
                            fillers = bgen
                        for i, as_ in enumerate(att_steps(hp_i - 1)):
                            f = next(fillers, None)
                            if f is not None:
                                f()
                            as_()
                            # only the v-hosting windows need the denser
                            # 1.5x pacing; build-only windows stay 1:1 so
                            # the build spreads across the whole window
                            if hp_i <= 4 and i % 2 == 1:
                                f = next(fillers, None)
                                if f is not None:
                                    f()
                        for f in fillers:
                            f()

                    # final pair's attention has no build to hide Exp under —
                    # interleave the ci 0..6 accumulation of output tiles 0
                    # AND 1 into it: tile 0 fills the two build banks, is
                    # drained early (bias folded in) to SBUF partials, and
                    # the banks are reused for tile 1, which stays resident
                    # for the tail
                    with tc.tile_pool(name="ot", bufs=3) as otp:
                        pe0 = [qkps.tile([128, 512], F32, tag="pq", name="pq")
                               for _ in range(2)]
                        sbp = [att.tile([128, 512], F32, tag=f"sbp{c}",
                                        name=f"sbp{c}", bufs=1)
                               for c in range(2)]

                        def proj_early_steps():
                            # one matmul per step: spreads the extra PE work
                            # across more of the Exp-paced attention steps
                            for cch in range(2):
                                for ci in range(CINT - 1):
                                    def step(ci=ci, cch=cch):
                                        nc.tensor.matmul(
                                            pe0[cch][:], yt[:, ci, 0:128],
                                            wpa[:, ci, ts(cch, 512)],
                                            start=(ci == 0),
                                            stop=(ci == CINT - 2),
                                            skip_group_check=True,
                                        )
                                    yield step
                            for cch in range(2):
                                def dstep(cch=cch):
                                    nc.vector.tensor_add(
                                        sbp[cch][:], pe0[cch][:],
                                        bo[:, ts(cch, 512)],
                                    )
                                yield dstep
                            for cch in range(2):
                                for ci in range(CINT - 1):
                                    def step2(ci=ci, cch=cch):
                                        nc.tensor.matmul(
                                            pe0[cch][:], yt[:, ci, 128:256],
                                            wpa[:, ci, ts(cch, 512)],
                                            start=(ci == 0), stop=False,
                                            skip_group_check=True,
                                        )
                                    yield step2

                        for as_, ps in zip_longest(att_steps(NPAIR - 1),
                                                   proj_early_steps()):
                            if as_ is not None:
                                as_()
                            if ps is not None:
                                ps()

                        # ---- phase 3: out = y @ W_p.T + b_p ----
                        # runs inside the attention pool scope, reusing the
                        # sps ring for PSUM: no pool-switch barrier anywhere.
                        # tile 2 first (its ci7 gives the pair-7 transpose
                        # time to land), then tiles 0/1 whose early partials
                        # only need their ci7 pass.
                        for tt in (2, 3):
                            pos = spsp.tile([128, 2, 512], F32, tag="sp",
                                            name="sp")
                            proj_tile(tt, lambda cch, pos=pos: pos[:, cch, :],
                                      otp)
                        pos0 = spsp.tile([128, 2, 512], F32, tag="sp",
                                         name="sp")
                        proj_tile(0, lambda cch: pos0[:, cch, :], otp,
                                  ci_lo=CINT - 1, start_ci=CINT - 1,
                                  bias_ap=lambda cch: sbp[cch][:])
                        proj_tile(1, lambda cch: pe0[cch][:], otp,
                                  ci_lo=CINT - 1)
                        for tt in range(4, TT - 1):
                            pos = spsp.tile([128, 2, 512], F32, tag="sp",
                                            name="sp")
                            proj_tile(tt, lambda cch, pos=pos: pos[:, cch, :],
                                      otp)
                        # last tile: three staggered groups (512/384/128
                        # cols) in independent PSUM tiles so the FINAL accum
                        # group is tiny -- its drain chain (add + store +
                        # fixed DMA completion constants) starts as early as
                        # possible. Separate staging tiles and three DMA
                        # queues keep the drains fully parallel.
                        tt_l = TT - 1
                        pl = [qkps.tile([128, 512], F32, tag="pq", name="pq")
                              for _ in range(3)]
                        spans = [(0, 512), (512, 800), (800, 1024)]
                        engs = [nc.gpsimd, nc.sync, nc.scalar]
                        for i in range(3):
                            lo, hi = spans[i]
                            for ci in range(CINT):
                                nc.tensor.matmul(
                                    pl[i][:, 0 : hi - lo],
                                    yt[:, ci, ts(tt_l, 128)],
                                    wpa[:, ci, lo:hi],
                                    start=(ci == 0), stop=(ci == CINT - 1),
                                )
                            oti = otp.tile([128, 512], F32, tag="otl",
                                           name="otl", bufs=3)
                            nc.vector.tensor_add(
                                oti[:, 0 : hi - lo], pl[i][:, 0 : hi - lo],
                                bo[:, lo:hi],
                            )
                            engs[i].dma_start(
                                out=out_d[ts(tt_l, 128), lo:hi],
                                in_=oti[:, 0 : hi - lo],
                            )

    nc.compile()
    return nc


_NC_CACHE = {}


def _in_maps(inputs):
    x = np.asarray(inputs["x"], dtype=np.float32)
    sid = np.asarray(inputs["subject_id"]).astype(np.int64)
    W_qkv = np.asarray(inputs["W_qkv"], dtype=np.float32)
    b_qkv = np.asarray(inputs["b_qkv"], dtype=np.float32)
    A1 = np.asarray(inputs["A1"], dtype=np.float32)
    B1 = np.asarray(inputs["B1"], dtype=np.float32)
    W_p = np.asarray(inputs["W_p"], dtype=np.float32)
    b_p = np.asarray(inputs["b_p"], dtype=np.float32)
    A2 = np.asarray(inputs["A2"], dtype=np.float32)
    B2 = np.asarray(inputs["B2"], dtype=np.float32)

    bqk = np.ascontiguousarray(b_qkv[:2048].reshape(H, 128).T)
    bv = np.ascontiguousarray(
        np.broadcast_to(b_qkv[2048:3072], (128, C)).astype(np.float32)
    )
    bo = np.ascontiguousarray(np.broadcast_to(b_p, (128, C)).astype(np.float32))

    in_maps = []
    for b in range(NCORES):
        s = int(sid[b])
        W1 = W_qkv + ALPHA_OVER_RANK * (B1[s] @ A1[s])
        Wp = W_p + ALPHA_OVER_RANK * (B2[s] @ A2[s])
        in_maps.append(
            {
                "xT": np.ascontiguousarray(x[b].T).astype(NPBF16),
                "wqkvT": np.ascontiguousarray(W1.T).astype(NPBF16),
                "wpT": np.ascontiguousarray(Wp.T).astype(NPBF16),
                "bqk": bqk,
                "bv": bv,
                "bo": bo,
            }
        )
    return in_maps


def kernel(**inputs):
    if "nc" not in _NC_CACHE:
        _NC_CACHE["nc"] = _build()
    nc = _NC_CACHE["nc"]

    res = run_bass_kernel_spmd(nc, _in_maps(inputs), core_ids=list(range(NCORES)))
    out = np.stack([r["out"] for r in res.results], axis=0)
    return out.astype(np.float32)

